# revision 20
# baseline (speedup 1.0000x reference)
"""AttentionalPropagation (SuperGlue-style GNN message passing) on 8 TRN2 NeuronCores.

Sharding: pure data parallel over the batch dim (B=8 -> one batch element per core).
Per-core computation (x, src are (256, 2048) slices; all matmuls in bf16, f32 accum):

  Q = WqS @ x + bq          (256, 2048)   stacked-head layout, c = h*64+dh
  K = WkS @ s + bk          (256, 2048)
  VT = s^T @ WvS^T + bv     (2048, 256)   keys on partitions (transposed layout)
  per head h: S^T[m,n] = K_h[:,m] . Q_h[:,n]  -> exp(S^T/8)  (no max-subtraction;
      scores are O(1) so exp is safe)
  msg_u[dh,n] = sum_m exp . VT[m, h*64+dh]  (col-packed head pairs)
  den[n] = sum_m exp                        (4-way col-packed ones-matmuls)
  msg = msg_u / den
  h1 = W1x @ x + (W1m@WmP) @ msg   (Wm folded into W1 on host; b1/bm-terms cancel
                                    in InstanceNorm)
  hn = relu(h1 - mean);  out = (W2 * rstd) @ hn + b2   (rstd>0 commutes with relu)

Scheduling: software-pipelined one n-chunk back AND interleaved at super-tile
granularity (scores for chunk j alternate with msg/den for chunk j-1 in the PE
stream), m-accumulation chains run reversed so Tile emits at most one semaphore
wait per chain.
"""

import os
import sys

for _p in ("/opt/trn_rl_repo",):
    if _p not in sys.path:
        sys.path.insert(0, _p)

import numpy as np
import ml_dtypes

import concourse.bass as bass
import concourse.mybir as mybir
from concourse import bacc
from concourse import library_config
from concourse.bass import ts
from concourse.tile import TileContext
from concourse.bass_utils import run_bass_kernel_spmd

F32 = mybir.dt.float32
BF16 = mybir.dt.bfloat16
AF = mybir.ActivationFunctionType
ALU = mybir.AluOpType

B, D, N, M, H, DH = 8, 256, 2048, 2048, 4, 64
EPS = 1e-5
NCH = 4  # n-chunks of 512
CHUNK = 512


def _build():
    nc = bacc.Bacc("TRN2", target_bir_lowering=False, debug=False, num_devices=8)

    x_d = nc.dram_tensor("x", [2, 128, N], BF16, kind="ExternalInput").ap()
    s_d = nc.dram_tensor("src", [2, 128, M], BF16, kind="ExternalInput").ap()
    wq_d = nc.dram_tensor("wqT", [2, 128, D], BF16, kind="ExternalInput").ap()
    wk_d = nc.dram_tensor("wkT", [2, 128, D], BF16, kind="ExternalInput").ap()
    wv_d = nc.dram_tensor("wvT", [2, 128, D], BF16, kind="ExternalInput").ap()
    w1_d = nc.dram_tensor("w1T", [4, 128, 2 * D], BF16, kind="ExternalInput").ap()
    w2_d = nc.dram_tensor("w2T", [4, 128, D], BF16, kind="ExternalInput").ap()
    # biases packed as columns: [bq, bk, b2]
    bias_d = nc.dram_tensor("bias", [2, 128, 3], F32, kind="ExternalInput").ap()
    bv_d = nc.dram_tensor("bv", [1, D], BF16, kind="ExternalInput").ap()
    out_d = nc.dram_tensor("out", [D, N], F32, kind="ExternalOutput").ap()

    with TileContext(nc) as tc:
        nc.gpsimd.load_library(library_config.attn)
        with (
            tc.tile_pool(name="const", bufs=1) as const,
            tc.tile_pool(name="data", bufs=1) as data,
            tc.tile_pool(name="reuse", bufs=2) as reuse,
            tc.tile_pool(name="exps", bufs=12) as exps,
            tc.tile_pool(name="small", bufs=2) as small,
            tc.tile_pool(name="msgn", bufs=4) as msgn,
            tc.tile_pool(name="ps_sc", bufs=2, space="PSUM") as ps_sc,
            tc.tile_pool(name="ps_shared", bufs=4, space="PSUM") as ps_shared,
        ):
            # ---- inputs + weights (few large DMAs; x/wq first for fast start) ----
            x_sb = data.tile([128, 2, N], BF16, name="x")
            wq_sb = const.tile([128, 2, D], BF16, name="wq")
            nc.sync.dma_start(out=x_sb[:], in_=x_d.rearrange("k p n -> p k n"))
            nc.sync.dma_start(out=wq_sb[:], in_=wq_d.rearrange("k p n -> p k n"))
            s_sb = reuse.tile([128, 2, M], BF16, name="s", tag="big")
            wk_sb = const.tile([128, 2, D], BF16, name="wk")
            wv_sb = const.tile([128, 2, D], BF16, name="wv")
            nc.sync.dma_start(out=s_sb[:], in_=s_d.rearrange("k p n -> p k n"))
            nc.sync.dma_start(out=wk_sb[:], in_=wk_d.rearrange("k p n -> p k n"))
            nc.sync.dma_start(out=wv_sb[:], in_=wv_d.rearrange("k p n -> p k n"))
            bias_sb = const.tile([128, 2, 3], F32, name="bias")
            nc.sync.dma_start(out=bias_sb[:], in_=bias_d.rearrange("k p n -> p k n"))
            bv_bc = const.tile([128, D], BF16, name="bvbc")
            bv_src = bass.AP(
                tensor=bv_d.tensor, offset=bv_d.offset, ap=[[0, 128]] + bv_d.ap[1:]
            )
            nc.sync.dma_start(out=bv_bc[:], in_=bv_src)
            w1_sb = const.tile([128, 4, 2 * D], BF16, name="w1")
            nc.sync.dma_start(out=w1_sb[:], in_=w1_d.rearrange("k p n -> p k n"))
            w2_sb = const.tile([128, 4, D], BF16, name="w2")
            nc.sync.dma_start(out=w2_sb[:], in_=w2_d.rearrange("k p n -> p k n"))
            eps_sb = const.tile([128, 1], F32, name="eps")
            nc.vector.memset(eps_sb[:], EPS)
            ones_sb = const.tile([128, 1], BF16, name="ones")
            nc.vector.memset(ones_sb[:], 1.0)

            # ---- QKV projections (weight-stationary: 1 LDW per 4 MMs) ----
            q_sb = data.tile([128, 2, N], BF16, name="q")
            k_sb = data.tile([128, 2, M], BF16, name="k")

            def emit_qk(c):
                for dst, w_sb, src_t, b_col in (
                    (q_sb, wq_sb, x_sb, 0),
                    (k_sb, wk_sb, s_sb, 1),
                ):
                    ps = [
                        ps_sc.tile([128, 2, CHUNK], F32, name="qk", tag="scps")
                        for _ in range(2)
                    ]
                    for k in range(2):
                        for j in range(NCH):
                            nc.tensor.matmul(
                                ps[j // 2][:, j % 2, :],
                                w_sb[:, k, ts(c, 128)],
                                src_t[:, k, ts(j, CHUNK)],
                                start=(k == 0),
                                stop=(k == 1),
                            )
                    for half in range(2):
                        nc.vector.tensor_scalar_add(
                            dst[:, c, ts(half, 2 * CHUNK)],
                            ps[half][:],
                            bias_sb[:, c, b_col : b_col + 1],
                        )

            # V^T: (m, c) layout, 65-wide per-head blocks with a ones column
            vT_sb = [data.tile([128, H, DH + 1], BF16, name=f"vT{t}")
                     for t in range(16)]

            def emit_vT():
                for t in range(16):
                    vp = ps_shared.tile([128, D], F32, name="vps", tag="sps")
                    for k in range(2):
                        nc.tensor.matmul(
                            vp[:],
                            s_sb[:, k, ts(t, 128)],
                            wv_sb[:, k, :],
                            start=(k == 0),
                            stop=(k == 1),
                        )
                    nc.vector.tensor_add(
                        vT_sb[t][:, :, 0:DH],
                        vp[:].rearrange("p (h d) -> p h d", h=H),
                        bv_bc[:].rearrange("p (h d) -> p h d", h=H),
                    )
                    nc.vector.memset(vT_sb[t][:, :, DH : DH + 1], 1.0)

            # ---- attention ----
            h1_sb = data.tile([128, 4, N], BF16, name="h1")
            stats_sb = data.tile([128, 4, NCH, 6], F32, name="stats")
            eS = {}  # (j, h, half) -> expS tile (128, 8, CHUNK)
            mn = {}  # (j, p) -> normalized msg pair tile (128, CHUNK)
            mps = {}  # (j, p) -> msg psum ; (j, 'd') -> den psum

            def emit_scores_super(j, p, s):
                # scores + exp for super-tile s (m-tiles 2s, 2s+1), head pair p
                if s % 4 == 0:
                    for h2 in range(2):
                        eS[(j, 2 * p + h2, s // 4)] = exps.tile(
                            [128, 8, CHUNK], BF16, name="expS", tag="expS"
                        )
                scp = [
                    ps_sc.tile([128, 2, CHUNK], F32, name="sc", tag="scps")
                    for _ in range(2)
                ]
                for jj in range(2):
                    mt = 2 * s + jj
                    for h2 in range(2):
                        nc.tensor.matmul(
                            scp[h2][:, jj, :],
                            k_sb[ts(h2, DH), p, ts(mt, 128)],
                            q_sb[ts(h2, DH), p, ts(j, CHUNK)],
                            start=True,
                            stop=True,
                        )
                for h2 in range(2):
                    nc.scalar.activation(
                        eS[(j, 2 * p + h2, s // 4)][
                            :, 2 * (s % 4) : 2 * (s % 4) + 2, :
                        ],
                        scp[h2][:],
                        AF.Exp,
                        scale=1.0 / 8.0,
                    )

            def emit_msg_head(j, h):
                # augmented-V msg chain (row 64 = denominator), reversed m so a
                # single wait at the chain head covers every exp dependency
                mps[(j, h)] = ps_shared.tile(
                    [DH + 1, CHUNK], F32, name="msgps", tag="sps"
                )
                for mi in range(16):
                    mt = 15 - mi
                    nc.tensor.matmul(
                        mps[(j, h)][:],
                        vT_sb[mt][:, h, :],
                        eS[(j, h, mt // 8)][:, mt % 8, :],
                        start=(mi == 0),
                        stop=(mi == 15),
                    )
                del eS[(j, h, 0)], eS[(j, h, 1)]

            def emit_norm_h1(j):
                for p in range(2):
                    mn[(j, p)] = msgn.tile([128, CHUNK], BF16, name="mn", tag="mn")
                for h in range(4):
                    p, h2 = h // 2, h % 2
                    mp = mps.pop((j, h))
                    den = small.tile([1, CHUNK], F32, name="den", tag="den")
                    nc.vector.tensor_copy(den[:], mp[DH : DH + 1, :])
                    rden = small.tile([1, CHUNK], F32, name="rden", tag="rden")
                    nc.vector.reciprocal_approx_fast(rden[:], den[:])
                    rbc = small.tile([DH, CHUNK], F32, name="rbc", tag="rbc")
                    nc.gpsimd.partition_broadcast(rbc[:], rden[:])
                    nc.vector.tensor_mul(
                        mn[(j, p)][ts(h2, DH), :], mp[0:DH, :], rbc[:]
                    )
                # h1 = W1x @ x + W1mWm @ msg (reversed K: one wait covers the chain)
                for o in range(4):
                    hp = ps_shared.tile([128, CHUNK], F32, name="h1ps", tag="sps")
                    for ki, k in enumerate((3, 2, 1, 0)):
                        rhs = (
                            x_sb[:, k, ts(j, CHUNK)] if k < 2 else mn[(j, k - 2)][:]
                        )
                        nc.tensor.matmul(
                            hp[:],
                            w1_sb[:, k, ts(o, 128)],
                            rhs,
                            start=(ki == 0),
                            stop=(ki == 3),
                        )
                    nc.vector.tensor_copy(h1_sb[:, o, ts(j, CHUNK)], hp[:])
                    nc.vector.bn_stats(
                        stats_sb[:, o, j, :], h1_sb[:, o, ts(j, CHUNK)]
                    )

            # ---- schedule ----
            emit_qk(0)
            for s in range(8):
                emit_scores_super(0, 0, s)
            emit_qk(1)
            for s in range(8):
                emit_scores_super(0, 1, s)
            emit_vT()
            for j in range(1, NCH):
                for s in range(8):
                    emit_scores_super(j, 0, s)
                    emit_scores_super(j, 1, s)
                for h in range(4):
                    emit_msg_head(j - 1, h)
                emit_norm_h1(j - 1)
            for h in range(4):
                emit_msg_head(NCH - 1, h)
            emit_norm_h1(NCH - 1)

            # ---- InstanceNorm (relu on DVE, rstd folded into W2) + W2 ----
            hn_sb = reuse.tile([128, 4, N], BF16, name="hn", tag="big")
            mean = small.tile([128, 4], F32, name="mean", tag="mean")
            for o in range(4):
                mv = small.tile([128, 2], F32, name="mv", tag="mv")
                nc.vector.bn_aggr(mv[:], stats_sb[:, o, :, :])
                nc.vector.tensor_copy(mean[:, o : o + 1], mv[:, 0:1])
                std = small.tile([128, 1], F32, name="std", tag="std")
                nc.scalar.activation(std[:], mv[:, 1:2], AF.Sqrt, bias=eps_sb[:])
                rstd = small.tile([128, 1], F32, name="rstd", tag="rstd")
                nc.vector.reciprocal(rstd[:], std[:])
                nc.vector.tensor_scalar_mul(w2_sb[:, o, :], w2_sb[:, o, :], rstd[:])
            for j in range(NCH):
                for o in range(4):
                    nc.vector.tensor_scalar(
                        hn_sb[:, o, ts(j, CHUNK)],
                        h1_sb[:, o, ts(j, CHUNK)],
                        mean[:, o : o + 1],
                        0.0,
                        op0=ALU.subtract,
                        op1=ALU.max,
                    )
                for c in range(2):
                    op = ps_shared.tile([128, CHUNK], F32, name="ops", tag="sps")
                    for k in range(4):
                        nc.tensor.matmul(
                            op[:],
                            w2_sb[:, k, ts(c, 128)],
                            hn_sb[:, k, ts(j, CHUNK)],
                            start=(k == 0),
                            stop=(k == 3),
                        )
                    ot = small.tile([128, CHUNK], F32, name="outt", tag="outt")
                    nc.vector.tensor_scalar_add(ot[:], op[:], bias_sb[:, c, 2:3])
                    nc.sync.dma_start(out=out_d[ts(c, 128), ts(j, CHUNK)], in_=ot[:])

    nc.compile()
    return nc


_NC = None


def _get_nc():
    global _NC
    if _NC is None:
        _NC = _build()
    return _NC


def kernel(**inputs):
    x = np.asarray(inputs["x"], np.float32)
    source = np.asarray(inputs["source"], np.float32)
    Wq = np.asarray(inputs["Wq"], np.float32)
    bq = np.asarray(inputs["bq"], np.float32)
    Wk = np.asarray(inputs["Wk"], np.float32)
    bk = np.asarray(inputs["bk"], np.float32)
    Wv = np.asarray(inputs["Wv"], np.float32)
    bv = np.asarray(inputs["bv"], np.float32)
    Wm = np.asarray(inputs["Wm"], np.float64)
    W1 = np.asarray(inputs["W1"], np.float64)
    W2 = np.asarray(inputs["W2"], np.float32)
    b2 = np.asarray(inputs["b2"], np.float32)

    bf = ml_dtypes.bfloat16
    wqT = np.ascontiguousarray(Wq.reshape(H * DH, D).T).astype(bf).reshape(2, 128, D)
    wkT = np.ascontiguousarray(Wk.reshape(H * DH, D).T).astype(bf).reshape(2, 128, D)
    wvT = np.ascontiguousarray(Wv.reshape(H * DH, D).T).astype(bf).reshape(2, 128, D)
    # message-channel permutation (dh-major -> head-major) folded into Wm
    WmP = Wm.reshape(D, DH, H).transpose(0, 2, 1).reshape(D, D)
    # fold Wm into W1's message half; b1 and W1m@bm cancel in InstanceNorm
    W1mWm = W1[:, D:] @ WmP
    w1T = (
        np.vstack([W1[:, :D].T, W1mWm.T])
        .astype(np.float32)
        .astype(bf)
        .reshape(4, 128, 2 * D)
    )
    w2T = np.ascontiguousarray(W2.T).astype(bf).reshape(4, 128, D)
    bias = np.stack(
        [bq.reshape(D).astype(np.float32), bk.reshape(D).astype(np.float32),
         b2.reshape(D)], axis=1
    ).reshape(2, 128, 3)
    shared = {
        "wqT": wqT,
        "wkT": wkT,
        "wvT": wvT,
        "w1T": np.ascontiguousarray(w1T),
        "w2T": w2T,
        "bias": np.ascontiguousarray(bias),
        "bv": np.ascontiguousarray(bv.reshape(1, D)).astype(bf),
    }
    in_maps = []
    for b in range(B):
        m = dict(shared)
        m["x"] = np.ascontiguousarray(x[b]).astype(bf).reshape(2, 128, N)
        m["src"] = np.ascontiguousarray(source[b]).astype(bf).reshape(2, 128, M)
        in_maps.append(m)

    nc = _get_nc()
    res = run_bass_kernel_spmd(nc, in_maps, core_ids=list(range(B)))
    return np.stack([res.results[b]["out"] for b in range(B)], axis=0)


# revision 21
# speedup vs baseline: 1.1105x; 1.1105x over previous
"""AttentionalPropagation (SuperGlue-style GNN message passing) on 8 TRN2 NeuronCores.

Sharding: pure data parallel over the batch dim (B=8 -> one batch element per core).
Per-core computation (x, src are (256, 2048) slices; all matmuls in bf16, f32 accum):

  Q = WqS @ x + bq          (256, 2048)   stacked-head layout, c = h*64+dh
  K = WkS @ s + bk          (256, 2048)
  VT = s^T @ WvS^T + bv     (2048, 256)   keys on partitions (transposed layout)
  per head h: S^T[m,n] = K_h[:,m] . Q_h[:,n]  -> exp(S^T/8)  (no max-subtraction;
      scores are O(1) so exp is safe)
  msg_u[dh,n] = sum_m exp . VT[m, h*64+dh]  (col-packed head pairs)
  den[n] = sum_m exp                        (4-way col-packed ones-matmuls)
  msg = msg_u / den
  h1 = W1x @ x + (W1m@WmP) @ msg   (Wm folded into W1 on host; b1/bm-terms cancel
                                    in InstanceNorm)
  hn = relu(h1 - mean);  out = (W2 * rstd) @ hn + b2   (rstd>0 commutes with relu)

Scheduling: software-pipelined one n-chunk back AND interleaved at super-tile
granularity (scores for chunk j alternate with msg/den for chunk j-1 in the PE
stream), m-accumulation chains run reversed so Tile emits at most one semaphore
wait per chain.
"""

import os
import sys

for _p in ("/opt/trn_rl_repo",):
    if _p not in sys.path:
        sys.path.insert(0, _p)

import numpy as np
import ml_dtypes

import concourse.bass as bass
import concourse.mybir as mybir
from concourse import bacc
from concourse import library_config
from concourse.bass import ts
from concourse.tile import TileContext
from concourse.bass_utils import run_bass_kernel_spmd

F32 = mybir.dt.float32
BF16 = mybir.dt.bfloat16
AF = mybir.ActivationFunctionType
ALU = mybir.AluOpType

B, D, N, M, H, DH = 8, 256, 2048, 2048, 4, 64
EPS = 1e-5
NCH = 4  # n-chunks of 512
CHUNK = 512


def _build():
    nc = bacc.Bacc("TRN2", target_bir_lowering=False, debug=False, num_devices=8)

    x_d = nc.dram_tensor("x", [2, 128, N], BF16, kind="ExternalInput").ap()
    s_d = nc.dram_tensor("src", [2, 128, M], BF16, kind="ExternalInput").ap()
    wq_d = nc.dram_tensor("wqT", [2, 128, D], BF16, kind="ExternalInput").ap()
    wk_d = nc.dram_tensor("wkT", [2, 128, D], BF16, kind="ExternalInput").ap()
    wv_d = nc.dram_tensor("wvT", [2, 128, D], BF16, kind="ExternalInput").ap()
    w1_d = nc.dram_tensor("w1T", [4, 128, 2 * D], BF16, kind="ExternalInput").ap()
    w2_d = nc.dram_tensor("w2T", [4, 128, D], BF16, kind="ExternalInput").ap()
    # biases packed as columns: [bq, bk, b2]
    bias_d = nc.dram_tensor("bias", [2, 128, 3], F32, kind="ExternalInput").ap()
    bv_d = nc.dram_tensor("bv", [1, D], BF16, kind="ExternalInput").ap()
    out_d = nc.dram_tensor("out", [D, N], F32, kind="ExternalOutput").ap()

    with TileContext(nc) as tc:
        nc.gpsimd.load_library(library_config.attn)
        with (
            tc.tile_pool(name="const", bufs=1) as const,
            tc.tile_pool(name="data", bufs=1) as data,
            tc.tile_pool(name="reuse", bufs=2) as reuse,
            tc.tile_pool(name="exps", bufs=6) as exps,
            tc.tile_pool(name="small", bufs=2) as small,
            tc.tile_pool(name="msgn", bufs=4) as msgn,
            tc.tile_pool(name="ps_sc", bufs=2, space="PSUM") as ps_sc,
            tc.tile_pool(name="ps_shared", bufs=4, space="PSUM") as ps_shared,
        ):
            # ---- inputs + weights (few large DMAs; x/wq first for fast start) ----
            x_sb = data.tile([128, 2, N], BF16, name="x")
            wq_sb = const.tile([128, 2, D], BF16, name="wq")
            nc.sync.dma_start(out=x_sb[:], in_=x_d.rearrange("k p n -> p k n"))
            nc.sync.dma_start(out=wq_sb[:], in_=wq_d.rearrange("k p n -> p k n"))
            s_sb = reuse.tile([128, 2, M], BF16, name="s", tag="big")
            wk_sb = const.tile([128, 2, D], BF16, name="wk")
            wv_sb = const.tile([128, 2, D], BF16, name="wv")
            nc.sync.dma_start(out=s_sb[:], in_=s_d.rearrange("k p n -> p k n"))
            nc.sync.dma_start(out=wk_sb[:], in_=wk_d.rearrange("k p n -> p k n"))
            nc.sync.dma_start(out=wv_sb[:], in_=wv_d.rearrange("k p n -> p k n"))
            bias_sb = const.tile([128, 2, 3], F32, name="bias")
            nc.sync.dma_start(out=bias_sb[:], in_=bias_d.rearrange("k p n -> p k n"))
            bv_bc = const.tile([128, D], BF16, name="bvbc")
            bv_src = bass.AP(
                tensor=bv_d.tensor, offset=bv_d.offset, ap=[[0, 128]] + bv_d.ap[1:]
            )
            nc.sync.dma_start(out=bv_bc[:], in_=bv_src)
            w1_sb = const.tile([128, 4, 2 * D], BF16, name="w1")
            nc.sync.dma_start(out=w1_sb[:], in_=w1_d.rearrange("k p n -> p k n"))
            w2_sb = const.tile([128, 4, D], BF16, name="w2")
            nc.sync.dma_start(out=w2_sb[:], in_=w2_d.rearrange("k p n -> p k n"))
            eps_sb = const.tile([128, 1], F32, name="eps")
            nc.vector.memset(eps_sb[:], EPS)
            ones_sb = const.tile([128, 1], BF16, name="ones")
            nc.vector.memset(ones_sb[:], 1.0)

            # ---- QKV projections (weight-stationary: 1 LDW per 4 MMs) ----
            q_sb = data.tile([128, 2, N], BF16, name="q")
            k_sb = data.tile([128, 2, M], BF16, name="k")

            def emit_qk(c):
                for dst, w_sb, src_t, b_col in (
                    (q_sb, wq_sb, x_sb, 0),
                    (k_sb, wk_sb, s_sb, 1),
                ):
                    ps = [
                        ps_sc.tile([128, 2, CHUNK], F32, name="qk", tag="scps")
                        for _ in range(2)
                    ]
                    for k in range(2):
                        for j in range(NCH):
                            nc.tensor.matmul(
                                ps[j // 2][:, j % 2, :],
                                w_sb[:, k, ts(c, 128)],
                                src_t[:, k, ts(j, CHUNK)],
                                start=(k == 0),
                                stop=(k == 1),
                            )
                    for half in range(2):
                        nc.vector.tensor_scalar_add(
                            dst[:, c, ts(half, 2 * CHUNK)],
                            ps[half][:],
                            bias_sb[:, c, b_col : b_col + 1],
                        )

            # V^T: (m, c) layout, 65-wide per-head blocks with a ones column
            vT_sb = [data.tile([128, H, DH + 1], BF16, name=f"vT{t}")
                     for t in range(16)]

            def emit_vT():
                for t in range(16):
                    vp = ps_shared.tile([128, D], F32, name="vps", tag="sps")
                    for k in range(2):
                        nc.tensor.matmul(
                            vp[:],
                            s_sb[:, k, ts(t, 128)],
                            wv_sb[:, k, :],
                            start=(k == 0),
                            stop=(k == 1),
                        )
                    nc.vector.tensor_add(
                        vT_sb[t][:, :, 0:DH],
                        vp[:].rearrange("p (h d) -> p h d", h=H),
                        bv_bc[:].rearrange("p (h d) -> p h d", h=H),
                    )
                    nc.vector.memset(vT_sb[t][:, :, DH : DH + 1], 1.0)

            # ---- attention ----
            h1_sb = data.tile([128, 4, N], BF16, name="h1")
            stats_sb = data.tile([128, 4, NCH, 6], F32, name="stats")
            eS = {}  # (j, h, half) -> expS tile (128, 8, CHUNK)
            mn = {}  # (j, p) -> normalized msg pair tile (128, CHUNK)
            mps = {}  # (j, p) -> msg psum ; (j, 'd') -> den psum

            def emit_scores_super(j, p, s):
                # scores + exp for super-tile s (m-tiles 2s, 2s+1), head pair p
                if s == 0:
                    for h2 in range(2):
                        eS[(j, 2 * p + h2)] = exps.tile(
                            [128, 16, CHUNK], BF16, name="expS", tag="expS"
                        )
                scp = [
                    ps_sc.tile([128, 2, CHUNK], F32, name="sc", tag="scps")
                    for _ in range(2)
                ]
                for jj in range(2):
                    mt = 2 * s + jj
                    for h2 in range(2):
                        nc.tensor.matmul(
                            scp[h2][:, jj, :],
                            k_sb[ts(h2, DH), p, ts(mt, 128)],
                            q_sb[ts(h2, DH), p, ts(j, CHUNK)],
                            start=True,
                            stop=True,
                        )
                for h2 in range(2):
                    nc.scalar.activation(
                        eS[(j, 2 * p + h2)][:, 2 * s : 2 * s + 2, :],
                        scp[h2][:],
                        AF.Exp,
                        scale=1.0 / 8.0,
                    )

            def emit_msg_head(j, h):
                # augmented-V msg chain (row 64 = denominator), reversed m so a
                # single wait at the chain head covers every exp dependency
                mps[(j, h)] = ps_shared.tile(
                    [DH + 1, CHUNK], F32, name="msgps", tag="sps"
                )
                for mt in range(16):
                    nc.tensor.matmul(
                        mps[(j, h)][:],
                        vT_sb[mt][:, h, :],
                        eS[(j, h)][:, mt, :],
                        start=(mt == 0),
                        stop=(mt == 15),
                    )
                del eS[(j, h)]

            def emit_norm_h1(j):
                for p in range(2):
                    mn[(j, p)] = msgn.tile([128, CHUNK], BF16, name="mn", tag="mn")
                for h in range(4):
                    p, h2 = h // 2, h % 2
                    mp = mps.pop((j, h))
                    den = small.tile([1, CHUNK], F32, name="den", tag="den")
                    nc.vector.tensor_copy(den[:], mp[DH : DH + 1, :])
                    rden = small.tile([1, CHUNK], F32, name="rden", tag="rden")
                    nc.vector.reciprocal_approx_fast(rden[:], den[:])
                    rbc = small.tile([DH, CHUNK], F32, name="rbc", tag="rbc")
                    nc.gpsimd.partition_broadcast(rbc[:], rden[:])
                    nc.vector.tensor_mul(
                        mn[(j, p)][ts(h2, DH), :], mp[0:DH, :], rbc[:]
                    )
                # h1 = W1x @ x + W1mWm @ msg (reversed K: one wait covers the chain)
                for o in range(4):
                    hp = ps_shared.tile([128, CHUNK], F32, name="h1ps", tag="sps")
                    for ki, k in enumerate((3, 2, 1, 0)):
                        rhs = (
                            x_sb[:, k, ts(j, CHUNK)] if k < 2 else mn[(j, k - 2)][:]
                        )
                        nc.tensor.matmul(
                            hp[:],
                            w1_sb[:, k, ts(o, 128)],
                            rhs,
                            start=(ki == 0),
                            stop=(ki == 3),
                        )
                    nc.vector.tensor_copy(h1_sb[:, o, ts(j, CHUNK)], hp[:])
                    nc.vector.bn_stats(
                        stats_sb[:, o, j, :], h1_sb[:, o, ts(j, CHUNK)]
                    )

            # ---- schedule ----
            emit_qk(0)
            for s in range(8):
                emit_scores_super(0, 0, s)
            emit_qk(1)
            for s in range(8):
                emit_scores_super(0, 1, s)
            emit_vT()
            for j in range(1, NCH):
                for s in range(8):
                    emit_scores_super(j, 0, s)
                    emit_scores_super(j, 1, s)
                for h in range(4):
                    emit_msg_head(j - 1, h)
                emit_norm_h1(j - 1)
            for h in range(4):
                emit_msg_head(NCH - 1, h)
            emit_norm_h1(NCH - 1)

            # ---- InstanceNorm (relu on DVE, rstd folded into W2) + W2 ----
            hn_sb = reuse.tile([128, 4, N], BF16, name="hn", tag="big")
            mean = small.tile([128, 4], F32, name="mean", tag="mean")
            for o in range(4):
                mv = small.tile([128, 2], F32, name="mv", tag="mv")
                nc.vector.bn_aggr(mv[:], stats_sb[:, o, :, :])
                nc.vector.tensor_copy(mean[:, o : o + 1], mv[:, 0:1])
                std = small.tile([128, 1], F32, name="std", tag="std")
                nc.scalar.activation(std[:], mv[:, 1:2], AF.Sqrt, bias=eps_sb[:])
                rstd = small.tile([128, 1], F32, name="rstd", tag="rstd")
                nc.vector.reciprocal(rstd[:], std[:])
                nc.vector.tensor_scalar_mul(w2_sb[:, o, :], w2_sb[:, o, :], rstd[:])
            for j in range(NCH):
                for o in range(4):
                    nc.vector.tensor_scalar(
                        hn_sb[:, o, ts(j, CHUNK)],
                        h1_sb[:, o, ts(j, CHUNK)],
                        mean[:, o : o + 1],
                        0.0,
                        op0=ALU.subtract,
                        op1=ALU.max,
                    )
                for c in range(2):
                    op = ps_shared.tile([128, CHUNK], F32, name="ops", tag="sps")
                    for k in range(4):
                        nc.tensor.matmul(
                            op[:],
                            w2_sb[:, k, ts(c, 128)],
                            hn_sb[:, k, ts(j, CHUNK)],
                            start=(k == 0),
                            stop=(k == 3),
                        )
                    ot = small.tile([128, CHUNK], F32, name="outt", tag="outt")
                    nc.vector.tensor_scalar_add(ot[:], op[:], bias_sb[:, c, 2:3])
                    nc.sync.dma_start(out=out_d[ts(c, 128), ts(j, CHUNK)], in_=ot[:])

    nc.compile()
    return nc


_NC = None


def _get_nc():
    global _NC
    if _NC is None:
        _NC = _build()
    return _NC


def kernel(**inputs):
    x = np.asarray(inputs["x"], np.float32)
    source = np.asarray(inputs["source"], np.float32)
    Wq = np.asarray(inputs["Wq"], np.float32)
    bq = np.asarray(inputs["bq"], np.float32)
    Wk = np.asarray(inputs["Wk"], np.float32)
    bk = np.asarray(inputs["bk"], np.float32)
    Wv = np.asarray(inputs["Wv"], np.float32)
    bv = np.asarray(inputs["bv"], np.float32)
    Wm = np.asarray(inputs["Wm"], np.float64)
    W1 = np.asarray(inputs["W1"], np.float64)
    W2 = np.asarray(inputs["W2"], np.float32)
    b2 = np.asarray(inputs["b2"], np.float32)

    bf = ml_dtypes.bfloat16
    wqT = np.ascontiguousarray(Wq.reshape(H * DH, D).T).astype(bf).reshape(2, 128, D)
    wkT = np.ascontiguousarray(Wk.reshape(H * DH, D).T).astype(bf).reshape(2, 128, D)
    wvT = np.ascontiguousarray(Wv.reshape(H * DH, D).T).astype(bf).reshape(2, 128, D)
    # message-channel permutation (dh-major -> head-major) folded into Wm
    WmP = Wm.reshape(D, DH, H).transpose(0, 2, 1).reshape(D, D)
    # fold Wm into W1's message half; b1 and W1m@bm cancel in InstanceNorm
    W1mWm = W1[:, D:] @ WmP
    w1T = (
        np.vstack([W1[:, :D].T, W1mWm.T])
        .astype(np.float32)
        .astype(bf)
        .reshape(4, 128, 2 * D)
    )
    w2T = np.ascontiguousarray(W2.T).astype(bf).reshape(4, 128, D)
    bias = np.stack(
        [bq.reshape(D).astype(np.float32), bk.reshape(D).astype(np.float32),
         b2.reshape(D)], axis=1
    ).reshape(2, 128, 3)
    shared = {
        "wqT": wqT,
        "wkT": wkT,
        "wvT": wvT,
        "w1T": np.ascontiguousarray(w1T),
        "w2T": w2T,
        "bias": np.ascontiguousarray(bias),
        "bv": np.ascontiguousarray(bv.reshape(1, D)).astype(bf),
    }
    in_maps = []
    for b in range(B):
        m = dict(shared)
        m["x"] = np.ascontiguousarray(x[b]).astype(bf).reshape(2, 128, N)
        m["src"] = np.ascontiguousarray(source[b]).astype(bf).reshape(2, 128, M)
        in_maps.append(m)

    nc = _get_nc()
    res = run_bass_kernel_spmd(nc, in_maps, core_ids=list(range(B)))
    return np.stack([res.results[b]["out"] for b in range(B)], axis=0)


# revision 22
# speedup vs baseline: 1.1619x; 1.0463x over previous
"""AttentionalPropagation (SuperGlue-style GNN message passing) on 8 TRN2 NeuronCores.

Sharding: pure data parallel over the batch dim (B=8 -> one batch element per core).
Per-core computation (x, src are (256, 2048) slices; all matmuls in bf16, f32 accum):

  Q = WqS @ x + bq          (256, 2048)   stacked-head layout, c = h*64+dh
  K = WkS @ s + bk          (256, 2048)
  VT = s^T @ WvS^T + bv     (2048, 256)   keys on partitions (transposed layout)
  per head h: S^T[m,n] = K_h[:,m] . Q_h[:,n]  -> exp(S^T/8)  (no max-subtraction;
      scores are O(1) so exp is safe)
  msg_u[dh,n] = sum_m exp . VT[m, h*64+dh]  (col-packed head pairs)
  den[n] = sum_m exp                        (4-way col-packed ones-matmuls)
  msg = msg_u / den
  h1 = W1x @ x + (W1m@WmP) @ msg   (Wm folded into W1 on host; b1/bm-terms cancel
                                    in InstanceNorm)
  hn = relu(h1 - mean);  out = (W2 * rstd) @ hn + b2   (rstd>0 commutes with relu)

Scheduling: software-pipelined one n-chunk back AND interleaved at super-tile
granularity (scores for chunk j alternate with msg/den for chunk j-1 in the PE
stream), m-accumulation chains run reversed so Tile emits at most one semaphore
wait per chain.
"""

import os
import sys

for _p in ("/opt/trn_rl_repo",):
    if _p not in sys.path:
        sys.path.insert(0, _p)

import numpy as np
import ml_dtypes

import concourse.bass as bass
import concourse.mybir as mybir
from concourse import bacc
from concourse import library_config
from concourse.bass import ts
from concourse.tile import TileContext
from concourse.bass_utils import run_bass_kernel_spmd

F32 = mybir.dt.float32
BF16 = mybir.dt.bfloat16
AF = mybir.ActivationFunctionType
ALU = mybir.AluOpType

B, D, N, M, H, DH = 8, 256, 2048, 2048, 4, 64
EPS = 1e-5
NCH = 4  # n-chunks of 512
CHUNK = 512


def _build():
    nc = bacc.Bacc("TRN2", target_bir_lowering=False, debug=False, num_devices=8)

    x_d = nc.dram_tensor("x", [2, 128, N], BF16, kind="ExternalInput").ap()
    s_d = nc.dram_tensor("src", [2, 128, M], BF16, kind="ExternalInput").ap()
    wq_d = nc.dram_tensor("wqT", [2, 128, D], BF16, kind="ExternalInput").ap()
    wk_d = nc.dram_tensor("wkT", [2, 128, D], BF16, kind="ExternalInput").ap()
    wv_d = nc.dram_tensor("wvT", [2, 128, D], BF16, kind="ExternalInput").ap()
    w1_d = nc.dram_tensor("w1T", [4, 128, 2 * D], BF16, kind="ExternalInput").ap()
    w2_d = nc.dram_tensor("w2T", [4, 128, D], BF16, kind="ExternalInput").ap()
    # biases packed as columns: [bq, bk, b2]
    bias_d = nc.dram_tensor("bias", [2, 128, 3], F32, kind="ExternalInput").ap()
    bv_d = nc.dram_tensor("bv", [1, D], BF16, kind="ExternalInput").ap()
    out_d = nc.dram_tensor("out", [D, N], F32, kind="ExternalOutput").ap()

    with TileContext(nc) as tc:
        nc.gpsimd.load_library(library_config.attn)
        with (
            tc.tile_pool(name="const", bufs=1) as const,
            tc.tile_pool(name="data", bufs=1) as data,
            tc.tile_pool(name="reuse", bufs=2) as reuse,
            tc.tile_pool(name="exps", bufs=6) as exps,
            tc.tile_pool(name="small", bufs=2) as small,
            tc.tile_pool(name="msgn", bufs=4) as msgn,
            tc.tile_pool(name="ps_sc", bufs=2, space="PSUM") as ps_sc,
            tc.tile_pool(name="ps_shared", bufs=4, space="PSUM") as ps_shared,
        ):
            # ---- inputs + weights (few large DMAs; x/wq first for fast start) ----
            x_sb = data.tile([128, 2, N], BF16, name="x")
            wq_sb = const.tile([128, 2, D], BF16, name="wq")
            nc.sync.dma_start(out=x_sb[:], in_=x_d.rearrange("k p n -> p k n"))
            nc.sync.dma_start(out=wq_sb[:], in_=wq_d.rearrange("k p n -> p k n"))
            s_sb = reuse.tile([128, 2, M], BF16, name="s", tag="big")
            wk_sb = const.tile([128, 2, D], BF16, name="wk")
            wv_sb = const.tile([128, 2, D], BF16, name="wv")
            nc.sync.dma_start(out=s_sb[:], in_=s_d.rearrange("k p n -> p k n"))
            nc.sync.dma_start(out=wk_sb[:], in_=wk_d.rearrange("k p n -> p k n"))
            nc.sync.dma_start(out=wv_sb[:], in_=wv_d.rearrange("k p n -> p k n"))
            bias_sb = const.tile([128, 2, 3], F32, name="bias")
            nc.sync.dma_start(out=bias_sb[:], in_=bias_d.rearrange("k p n -> p k n"))
            bv_bc = const.tile([128, D], BF16, name="bvbc")
            bv_src = bass.AP(
                tensor=bv_d.tensor, offset=bv_d.offset, ap=[[0, 128]] + bv_d.ap[1:]
            )
            nc.sync.dma_start(out=bv_bc[:], in_=bv_src)
            w1_sb = const.tile([128, 4, 2 * D], BF16, name="w1")
            nc.sync.dma_start(out=w1_sb[:], in_=w1_d.rearrange("k p n -> p k n"))
            w2_sb = const.tile([128, 4, D], BF16, name="w2")
            nc.sync.dma_start(out=w2_sb[:], in_=w2_d.rearrange("k p n -> p k n"))
            eps_sb = const.tile([128, 1], F32, name="eps")
            nc.vector.memset(eps_sb[:], EPS)
            ones_sb = const.tile([128, 1], BF16, name="ones")
            nc.vector.memset(ones_sb[:], 1.0)

            # ---- QKV projections (weight-stationary: 1 LDW per 4 MMs) ----
            q_sb = data.tile([128, 2, N], BF16, name="q")
            k_sb = data.tile([128, 2, M], BF16, name="k")

            def emit_qk(c):
                for dst, w_sb, src_t, b_col in (
                    (q_sb, wq_sb, x_sb, 0),
                    (k_sb, wk_sb, s_sb, 1),
                ):
                    ps = [
                        ps_sc.tile([128, 2, CHUNK], F32, name="qk", tag="scps")
                        for _ in range(2)
                    ]
                    for k in range(2):
                        for j in range(NCH):
                            nc.tensor.matmul(
                                ps[j // 2][:, j % 2, :],
                                w_sb[:, k, ts(c, 128)],
                                src_t[:, k, ts(j, CHUNK)],
                                start=(k == 0),
                                stop=(k == 1),
                            )
                    for half in range(2):
                        nc.vector.tensor_scalar_add(
                            dst[:, c, ts(half, 2 * CHUNK)],
                            ps[half][:],
                            bias_sb[:, c, b_col : b_col + 1],
                        )

            # V^T: (m, c) layout, 65-wide per-head blocks with a ones column
            vT_sb = [data.tile([128, H, DH + 1], BF16, name=f"vT{t}")
                     for t in range(16)]

            def emit_vT():
                for t in range(16):
                    vp = ps_shared.tile([128, D], F32, name="vps", tag="sps")
                    for k in range(2):
                        nc.tensor.matmul(
                            vp[:],
                            s_sb[:, k, ts(t, 128)],
                            wv_sb[:, k, :],
                            start=(k == 0),
                            stop=(k == 1),
                        )
                    nc.vector.tensor_add(
                        vT_sb[t][:, :, 0:DH],
                        vp[:].rearrange("p (h d) -> p h d", h=H),
                        bv_bc[:].rearrange("p (h d) -> p h d", h=H),
                    )
                    nc.vector.memset(vT_sb[t][:, :, DH : DH + 1], 1.0)

            # ---- attention ----
            h1_sb = data.tile([128, 4, N], BF16, name="h1")
            stats_sb = data.tile([128, 4, NCH, 6], F32, name="stats")
            eS = {}  # (j, h, half) -> expS tile (128, 8, CHUNK)
            mn = {}  # (j, p) -> normalized msg pair tile (128, CHUNK)
            mps = {}  # (j, p) -> msg psum ; (j, 'd') -> den psum

            def emit_scores_super(j, p, s):
                # scores + exp for super-tile s (m-tiles 2s, 2s+1), head pair p
                if s == 0:
                    for h2 in range(2):
                        eS[(j, 2 * p + h2)] = exps.tile(
                            [128, 16, CHUNK], BF16, name="expS", tag="expS"
                        )
                scp = [
                    ps_sc.tile([128, 2, CHUNK], F32, name="sc", tag="scps")
                    for _ in range(2)
                ]
                for jj in range(2):
                    mt = 2 * s + jj
                    for h2 in range(2):
                        nc.tensor.matmul(
                            scp[h2][:, jj, :],
                            k_sb[ts(h2, DH), p, ts(mt, 128)],
                            q_sb[ts(h2, DH), p, ts(j, CHUNK)],
                            start=True,
                            stop=True,
                        )
                for h2 in range(2):
                    nc.scalar.activation(
                        eS[(j, 2 * p + h2)][:, 2 * s : 2 * s + 2, :],
                        scp[h2][:],
                        AF.Exp,
                        scale=1.0 / 8.0,
                    )

            def emit_msg_head(j, h):
                # augmented-V msg chain (psum row 64 = denominator)
                p, h2 = h // 2, h % 2
                if h2 == 0:
                    mn[(j, p)] = msgn.tile([128, CHUNK], BF16, name="mn", tag="mn")
                mp = ps_shared.tile([DH + 1, CHUNK], F32, name="msgps", tag="sps")
                for mt in range(16):
                    nc.tensor.matmul(
                        mp[:],
                        vT_sb[mt][:, h, :],
                        eS[(j, h)][:, mt, :],
                        start=(mt == 0),
                        stop=(mt == 15),
                    )
                del eS[(j, h)]
                den = small.tile([1, CHUNK], F32, name="den", tag="den")
                nc.vector.tensor_copy(den[:], mp[DH : DH + 1, :])
                rden = small.tile([1, CHUNK], F32, name="rden", tag="rden")
                nc.vector.reciprocal_approx_fast(rden[:], den[:])
                rbc = small.tile([DH, CHUNK], F32, name="rbc", tag="rbc")
                nc.gpsimd.partition_broadcast(rbc[:], rden[:])
                nc.vector.tensor_mul(mn[(j, p)][ts(h2, DH), :], mp[0:DH, :], rbc[:])

            def emit_norm_h1(j):
                # h1 = W1x @ x + W1mWm @ msg
                for o in range(4):
                    hp = ps_shared.tile([128, CHUNK], F32, name="h1ps", tag="sps")
                    for k in range(4):
                        rhs = (
                            x_sb[:, k, ts(j, CHUNK)] if k < 2 else mn[(j, k - 2)][:]
                        )
                        nc.tensor.matmul(
                            hp[:],
                            w1_sb[:, k, ts(o, 128)],
                            rhs,
                            start=(k == 0),
                            stop=(k == 3),
                        )
                    nc.vector.tensor_copy(h1_sb[:, o, ts(j, CHUNK)], hp[:])
                    nc.vector.bn_stats(
                        stats_sb[:, o, j, :], h1_sb[:, o, ts(j, CHUNK)]
                    )

            # ---- schedule ----
            emit_qk(0)
            for s in range(8):
                emit_scores_super(0, 0, s)
            emit_qk(1)
            for s in range(8):
                emit_scores_super(0, 1, s)
            emit_vT()
            for j in range(1, NCH):
                for s in range(8):
                    emit_scores_super(j, 0, s)
                    emit_scores_super(j, 1, s)
                for h in range(4):
                    emit_msg_head(j - 1, h)
                emit_norm_h1(j - 1)
            for h in range(4):
                emit_msg_head(NCH - 1, h)
            emit_norm_h1(NCH - 1)

            # ---- InstanceNorm (relu on DVE, rstd folded into W2) + W2 ----
            hn_sb = reuse.tile([128, 4, N], BF16, name="hn", tag="big")
            mean = small.tile([128, 4], F32, name="mean", tag="mean")
            for o in range(4):
                mv = small.tile([128, 2], F32, name="mv", tag="mv")
                nc.vector.bn_aggr(mv[:], stats_sb[:, o, :, :])
                nc.vector.tensor_copy(mean[:, o : o + 1], mv[:, 0:1])
                std = small.tile([128, 1], F32, name="std", tag="std")
                nc.scalar.activation(std[:], mv[:, 1:2], AF.Sqrt, bias=eps_sb[:])
                rstd = small.tile([128, 1], F32, name="rstd", tag="rstd")
                nc.vector.reciprocal(rstd[:], std[:])
                nc.vector.tensor_scalar_mul(w2_sb[:, o, :], w2_sb[:, o, :], rstd[:])
            for j in range(NCH):
                for o in range(4):
                    nc.vector.tensor_scalar(
                        hn_sb[:, o, ts(j, CHUNK)],
                        h1_sb[:, o, ts(j, CHUNK)],
                        mean[:, o : o + 1],
                        0.0,
                        op0=ALU.subtract,
                        op1=ALU.max,
                    )
                for c in range(2):
                    op = ps_shared.tile([128, CHUNK], F32, name="ops", tag="sps")
                    for k in range(4):
                        nc.tensor.matmul(
                            op[:],
                            w2_sb[:, k, ts(c, 128)],
                            hn_sb[:, k, ts(j, CHUNK)],
                            start=(k == 0),
                            stop=(k == 3),
                        )
                    ot = small.tile([128, CHUNK], F32, name="outt", tag="outt")
                    nc.vector.tensor_scalar_add(ot[:], op[:], bias_sb[:, c, 2:3])
                    nc.sync.dma_start(out=out_d[ts(c, 128), ts(j, CHUNK)], in_=ot[:])

    nc.compile()
    return nc


_NC = None


def _get_nc():
    global _NC
    if _NC is None:
        _NC = _build()
    return _NC


def kernel(**inputs):
    x = np.asarray(inputs["x"], np.float32)
    source = np.asarray(inputs["source"], np.float32)
    Wq = np.asarray(inputs["Wq"], np.float32)
    bq = np.asarray(inputs["bq"], np.float32)
    Wk = np.asarray(inputs["Wk"], np.float32)
    bk = np.asarray(inputs["bk"], np.float32)
    Wv = np.asarray(inputs["Wv"], np.float32)
    bv = np.asarray(inputs["bv"], np.float32)
    Wm = np.asarray(inputs["Wm"], np.float64)
    W1 = np.asarray(inputs["W1"], np.float64)
    W2 = np.asarray(inputs["W2"], np.float32)
    b2 = np.asarray(inputs["b2"], np.float32)

    bf = ml_dtypes.bfloat16
    wqT = np.ascontiguousarray(Wq.reshape(H * DH, D).T).astype(bf).reshape(2, 128, D)
    wkT = np.ascontiguousarray(Wk.reshape(H * DH, D).T).astype(bf).reshape(2, 128, D)
    wvT = np.ascontiguousarray(Wv.reshape(H * DH, D).T).astype(bf).reshape(2, 128, D)
    # message-channel permutation (dh-major -> head-major) folded into Wm
    WmP = Wm.reshape(D, DH, H).transpose(0, 2, 1).reshape(D, D)
    # fold Wm into W1's message half; b1 and W1m@bm cancel in InstanceNorm
    W1mWm = W1[:, D:] @ WmP
    w1T = (
        np.vstack([W1[:, :D].T, W1mWm.T])
        .astype(np.float32)
        .astype(bf)
        .reshape(4, 128, 2 * D)
    )
    w2T = np.ascontiguousarray(W2.T).astype(bf).reshape(4, 128, D)
    bias = np.stack(
        [bq.reshape(D).astype(np.float32), bk.reshape(D).astype(np.float32),
         b2.reshape(D)], axis=1
    ).reshape(2, 128, 3)
    shared = {
        "wqT": wqT,
        "wkT": wkT,
        "wvT": wvT,
        "w1T": np.ascontiguousarray(w1T),
        "w2T": w2T,
        "bias": np.ascontiguousarray(bias),
        "bv": np.ascontiguousarray(bv.reshape(1, D)).astype(bf),
    }
    in_maps = []
    for b in range(B):
        m = dict(shared)
        m["x"] = np.ascontiguousarray(x[b]).astype(bf).reshape(2, 128, N)
        m["src"] = np.ascontiguousarray(source[b]).astype(bf).reshape(2, 128, M)
        in_maps.append(m)

    nc = _get_nc()
    res = run_bass_kernel_spmd(nc, in_maps, core_ids=list(range(B)))
    return np.stack([res.results[b]["out"] for b in range(B)], axis=0)


# revision 24
# speedup vs baseline: 1.1634x; 1.0013x over previous
"""AttentionalPropagation (SuperGlue-style GNN message passing) on 8 TRN2 NeuronCores.

Sharding: pure data parallel over the batch dim (B=8 -> one batch element per core).
Per-core computation (x, src are (256, 2048) slices; all matmuls in bf16, f32 accum):

  Q = WqS @ x + bq          (256, 2048)   stacked-head layout, c = h*64+dh
  K = WkS @ s + bk          (256, 2048)
  VT = s^T @ WvS^T + bv     (2048, 256)   keys on partitions (transposed layout)
  per head h: S^T[m,n] = K_h[:,m] . Q_h[:,n]  -> exp(S^T/8)  (no max-subtraction;
      scores are O(1) so exp is safe)
  msg_u[dh,n] = sum_m exp . VT[m, h*64+dh]  (col-packed head pairs)
  den[n] = sum_m exp                        (4-way col-packed ones-matmuls)
  msg = msg_u / den
  h1 = W1x @ x + (W1m@WmP) @ msg   (Wm folded into W1 on host; b1/bm-terms cancel
                                    in InstanceNorm)
  hn = relu(h1 - mean);  out = (W2 * rstd) @ hn + b2   (rstd>0 commutes with relu)

Scheduling: software-pipelined one n-chunk back AND interleaved at super-tile
granularity (scores for chunk j alternate with msg/den for chunk j-1 in the PE
stream), m-accumulation chains run reversed so Tile emits at most one semaphore
wait per chain.
"""

import os
import sys

for _p in ("/opt/trn_rl_repo",):
    if _p not in sys.path:
        sys.path.insert(0, _p)

import numpy as np
import ml_dtypes

import concourse.bass as bass
import concourse.mybir as mybir
from concourse import bacc
from concourse import library_config
from concourse.bass import ts
from concourse.tile import TileContext
from concourse.bass_utils import run_bass_kernel_spmd

F32 = mybir.dt.float32
BF16 = mybir.dt.bfloat16
AF = mybir.ActivationFunctionType
ALU = mybir.AluOpType

B, D, N, M, H, DH = 8, 256, 2048, 2048, 4, 64
EPS = 1e-5
NCH = 4  # n-chunks of 512
CHUNK = 512


def _build():
    nc = bacc.Bacc("TRN2", target_bir_lowering=False, debug=False, num_devices=8)

    x_d = nc.dram_tensor("x", [2, 128, N], BF16, kind="ExternalInput").ap()
    s_d = nc.dram_tensor("src", [2, 128, M], BF16, kind="ExternalInput").ap()
    wq_d = nc.dram_tensor("wqT", [2, 128, D], BF16, kind="ExternalInput").ap()
    wk_d = nc.dram_tensor("wkT", [2, 128, D], BF16, kind="ExternalInput").ap()
    wv_d = nc.dram_tensor("wvT", [2, 128, D], BF16, kind="ExternalInput").ap()
    w1_d = nc.dram_tensor("w1T", [4, 128, 2 * D], BF16, kind="ExternalInput").ap()
    w2_d = nc.dram_tensor("w2T", [4, 128, D], BF16, kind="ExternalInput").ap()
    # biases packed as columns: [bq, bk, b2]
    bias_d = nc.dram_tensor("bias", [2, 128, 3], F32, kind="ExternalInput").ap()
    bv_d = nc.dram_tensor("bv", [1, D], BF16, kind="ExternalInput").ap()
    out_d = nc.dram_tensor("out", [D, N], F32, kind="ExternalOutput").ap()

    with TileContext(nc) as tc:
        nc.gpsimd.load_library(library_config.attn)
        with (
            tc.tile_pool(name="const", bufs=1) as const,
            tc.tile_pool(name="data", bufs=1) as data,
            tc.tile_pool(name="reuse", bufs=2) as reuse,
            tc.tile_pool(name="exps", bufs=6) as exps,
            tc.tile_pool(name="small", bufs=2) as small,
            tc.tile_pool(name="msgn", bufs=4) as msgn,
            tc.tile_pool(name="ps_sc", bufs=2, space="PSUM") as ps_sc,
            tc.tile_pool(name="ps_shared", bufs=4, space="PSUM") as ps_shared,
        ):
            # ---- inputs + weights (few large DMAs; x/wq first for fast start) ----
            x_sb = data.tile([128, 2, N], BF16, name="x")
            wq_sb = const.tile([128, 2, D], BF16, name="wq")
            nc.sync.dma_start(out=x_sb[:], in_=x_d.rearrange("k p n -> p k n"))
            nc.sync.dma_start(out=wq_sb[:], in_=wq_d.rearrange("k p n -> p k n"))
            s_sb = reuse.tile([128, 2, M], BF16, name="s", tag="big")
            wk_sb = const.tile([128, 2, D], BF16, name="wk")
            wv_sb = const.tile([128, 2, D], BF16, name="wv")
            nc.gpsimd.dma_start(out=s_sb[:], in_=s_d.rearrange("k p n -> p k n"))
            nc.gpsimd.dma_start(out=wk_sb[:], in_=wk_d.rearrange("k p n -> p k n"))
            nc.gpsimd.dma_start(out=wv_sb[:], in_=wv_d.rearrange("k p n -> p k n"))
            bias_sb = const.tile([128, 2, 3], F32, name="bias")
            nc.scalar.dma_start(out=bias_sb[:], in_=bias_d.rearrange("k p n -> p k n"))
            bv_bc = const.tile([128, D], BF16, name="bvbc")
            bv_src = bass.AP(
                tensor=bv_d.tensor, offset=bv_d.offset, ap=[[0, 128]] + bv_d.ap[1:]
            )
            nc.scalar.dma_start(out=bv_bc[:], in_=bv_src)
            w1_sb = const.tile([128, 4, 2 * D], BF16, name="w1")
            nc.scalar.dma_start(out=w1_sb[:], in_=w1_d.rearrange("k p n -> p k n"))
            w2_sb = const.tile([128, 4, D], BF16, name="w2")
            nc.scalar.dma_start(out=w2_sb[:], in_=w2_d.rearrange("k p n -> p k n"))
            eps_sb = const.tile([128, 1], F32, name="eps")
            nc.vector.memset(eps_sb[:], EPS)
            ones_sb = const.tile([128, 1], BF16, name="ones")
            nc.vector.memset(ones_sb[:], 1.0)

            # ---- QKV projections (weight-stationary: 1 LDW per 4 MMs) ----
            q_sb = data.tile([128, 2, N], BF16, name="q")
            k_sb = data.tile([128, 2, M], BF16, name="k")

            def emit_qk(c):
                for dst, w_sb, src_t, b_col in (
                    (q_sb, wq_sb, x_sb, 0),
                    (k_sb, wk_sb, s_sb, 1),
                ):
                    ps = [
                        ps_sc.tile([128, 2, CHUNK], F32, name="qk", tag="scps")
                        for _ in range(2)
                    ]
                    for k in range(2):
                        for j in range(NCH):
                            nc.tensor.matmul(
                                ps[j // 2][:, j % 2, :],
                                w_sb[:, k, ts(c, 128)],
                                src_t[:, k, ts(j, CHUNK)],
                                start=(k == 0),
                                stop=(k == 1),
                            )
                    for half in range(2):
                        nc.vector.tensor_scalar_add(
                            dst[:, c, ts(half, 2 * CHUNK)],
                            ps[half][:],
                            bias_sb[:, c, b_col : b_col + 1],
                        )

            # V^T: (m, c) layout, 65-wide per-head blocks with a ones column
            vT_sb = [data.tile([128, H, DH + 1], BF16, name=f"vT{t}")
                     for t in range(16)]

            def emit_vT():
                for t in range(16):
                    vp = ps_shared.tile([128, D], F32, name="vps", tag="sps")
                    for k in range(2):
                        nc.tensor.matmul(
                            vp[:],
                            s_sb[:, k, ts(t, 128)],
                            wv_sb[:, k, :],
                            start=(k == 0),
                            stop=(k == 1),
                        )
                    nc.vector.tensor_add(
                        vT_sb[t][:, :, 0:DH],
                        vp[:].rearrange("p (h d) -> p h d", h=H),
                        bv_bc[:].rearrange("p (h d) -> p h d", h=H),
                    )
                    nc.vector.memset(vT_sb[t][:, :, DH : DH + 1], 1.0)

            # ---- attention ----
            h1_sb = data.tile([128, 4, N], BF16, name="h1")
            stats_sb = data.tile([128, 4, NCH, 6], F32, name="stats")
            eS = {}  # (j, h, half) -> expS tile (128, 8, CHUNK)
            mn = {}  # (j, p) -> normalized msg pair tile (128, CHUNK)
            mps = {}  # (j, p) -> msg psum ; (j, 'd') -> den psum

            def emit_scores_super(j, p, s):
                # scores + exp for super-tile s (m-tiles 2s, 2s+1), head pair p
                if s == 0:
                    for h2 in range(2):
                        eS[(j, 2 * p + h2)] = exps.tile(
                            [128, 16, CHUNK], BF16, name="expS", tag="expS"
                        )
                scp = [
                    ps_sc.tile([128, 2, CHUNK], F32, name="sc", tag="scps")
                    for _ in range(2)
                ]
                for jj in range(2):
                    mt = 2 * s + jj
                    for h2 in range(2):
                        nc.tensor.matmul(
                            scp[h2][:, jj, :],
                            k_sb[ts(h2, DH), p, ts(mt, 128)],
                            q_sb[ts(h2, DH), p, ts(j, CHUNK)],
                            start=True,
                            stop=True,
                        )
                for h2 in range(2):
                    nc.scalar.activation(
                        eS[(j, 2 * p + h2)][:, 2 * s : 2 * s + 2, :],
                        scp[h2][:],
                        AF.Exp,
                        scale=1.0 / 8.0,
                    )

            def emit_msg_head(j, h):
                # augmented-V msg chain (psum row 64 = denominator)
                p, h2 = h // 2, h % 2
                if h2 == 0:
                    mn[(j, p)] = msgn.tile([128, CHUNK], BF16, name="mn", tag="mn")
                mp = ps_shared.tile([DH + 1, CHUNK], F32, name="msgps", tag="sps")
                for mt in range(16):
                    nc.tensor.matmul(
                        mp[:],
                        vT_sb[mt][:, h, :],
                        eS[(j, h)][:, mt, :],
                        start=(mt == 0),
                        stop=(mt == 15),
                    )
                del eS[(j, h)]
                den = small.tile([1, CHUNK], F32, name="den", tag="den")
                nc.vector.tensor_copy(den[:], mp[DH : DH + 1, :])
                rden = small.tile([1, CHUNK], F32, name="rden", tag="rden")
                nc.vector.reciprocal_approx_fast(rden[:], den[:])
                rbc = small.tile([DH, CHUNK], F32, name="rbc", tag="rbc")
                nc.gpsimd.partition_broadcast(rbc[:], rden[:])
                nc.vector.tensor_mul(mn[(j, p)][ts(h2, DH), :], mp[0:DH, :], rbc[:])

            def emit_norm_h1(j):
                # h1 = W1x @ x + W1mWm @ msg
                for o in range(4):
                    hp = ps_shared.tile([128, CHUNK], F32, name="h1ps", tag="sps")
                    for k in range(4):
                        rhs = (
                            x_sb[:, k, ts(j, CHUNK)] if k < 2 else mn[(j, k - 2)][:]
                        )
                        nc.tensor.matmul(
                            hp[:],
                            w1_sb[:, k, ts(o, 128)],
                            rhs,
                            start=(k == 0),
                            stop=(k == 3),
                        )
                    nc.vector.tensor_copy(h1_sb[:, o, ts(j, CHUNK)], hp[:])
                    nc.vector.bn_stats(
                        stats_sb[:, o, j, :], h1_sb[:, o, ts(j, CHUNK)]
                    )

            # ---- schedule ----
            emit_qk(0)
            for s in range(8):
                emit_scores_super(0, 0, s)
            emit_qk(1)
            for s in range(8):
                emit_scores_super(0, 1, s)
            emit_vT()
            for j in range(1, NCH - 1):
                for s in range(8):
                    emit_scores_super(j, 0, s)
                    emit_scores_super(j, 1, s)
                for h in range(4):
                    emit_msg_head(j - 1, h)
                emit_norm_h1(j - 1)
            jL = NCH - 1
            for s in range(8):
                emit_scores_super(jL, 0, s)
            for s in range(8):
                emit_scores_super(jL, 1, s)
            for h in range(4):
                emit_msg_head(jL - 1, h)
            emit_norm_h1(jL - 1)
            for h in range(4):
                emit_msg_head(jL, h)
            emit_norm_h1(jL)

            # ---- InstanceNorm (relu on DVE, rstd folded into W2) + W2 ----
            hn_sb = reuse.tile([128, 4, N], BF16, name="hn", tag="big")
            mean = small.tile([128, 4], F32, name="mean", tag="mean")
            for o in range(4):
                mv = small.tile([128, 2], F32, name="mv", tag="mv")
                nc.vector.bn_aggr(mv[:], stats_sb[:, o, :, :])
                nc.vector.tensor_copy(mean[:, o : o + 1], mv[:, 0:1])
                std = small.tile([128, 1], F32, name="std", tag="std")
                nc.scalar.activation(std[:], mv[:, 1:2], AF.Sqrt, bias=eps_sb[:])
                rstd = small.tile([128, 1], F32, name="rstd", tag="rstd")
                nc.vector.reciprocal(rstd[:], std[:])
                nc.vector.tensor_scalar_mul(w2_sb[:, o, :], w2_sb[:, o, :], rstd[:])
            for j in range(NCH):
                for o in range(4):
                    nc.vector.tensor_scalar(
                        hn_sb[:, o, ts(j, CHUNK)],
                        h1_sb[:, o, ts(j, CHUNK)],
                        mean[:, o : o + 1],
                        0.0,
                        op0=ALU.subtract,
                        op1=ALU.max,
                    )
                for c in range(2):
                    op = ps_shared.tile([128, CHUNK], F32, name="ops", tag="sps")
                    for k in range(4):
                        nc.tensor.matmul(
                            op[:],
                            w2_sb[:, k, ts(c, 128)],
                            hn_sb[:, k, ts(j, CHUNK)],
                            start=(k == 0),
                            stop=(k == 3),
                        )
                    ot = small.tile([128, CHUNK], F32, name="outt", tag="outt")
                    nc.vector.tensor_scalar_add(ot[:], op[:], bias_sb[:, c, 2:3])
                    nc.sync.dma_start(out=out_d[ts(c, 128), ts(j, CHUNK)], in_=ot[:])

    nc.compile()
    return nc


_NC = None


def _get_nc():
    global _NC
    if _NC is None:
        _NC = _build()
    return _NC


def kernel(**inputs):
    x = np.asarray(inputs["x"], np.float32)
    source = np.asarray(inputs["source"], np.float32)
    Wq = np.asarray(inputs["Wq"], np.float32)
    bq = np.asarray(inputs["bq"], np.float32)
    Wk = np.asarray(inputs["Wk"], np.float32)
    bk = np.asarray(inputs["bk"], np.float32)
    Wv = np.asarray(inputs["Wv"], np.float32)
    bv = np.asarray(inputs["bv"], np.float32)
    Wm = np.asarray(inputs["Wm"], np.float64)
    W1 = np.asarray(inputs["W1"], np.float64)
    W2 = np.asarray(inputs["W2"], np.float32)
    b2 = np.asarray(inputs["b2"], np.float32)

    bf = ml_dtypes.bfloat16
    wqT = np.ascontiguousarray(Wq.reshape(H * DH, D).T).astype(bf).reshape(2, 128, D)
    wkT = np.ascontiguousarray(Wk.reshape(H * DH, D).T).astype(bf).reshape(2, 128, D)
    wvT = np.ascontiguousarray(Wv.reshape(H * DH, D).T).astype(bf).reshape(2, 128, D)
    # message-channel permutation (dh-major -> head-major) folded into Wm
    WmP = Wm.reshape(D, DH, H).transpose(0, 2, 1).reshape(D, D)
    # fold Wm into W1's message half; b1 and W1m@bm cancel in InstanceNorm
    W1mWm = W1[:, D:] @ WmP
    w1T = (
        np.vstack([W1[:, :D].T, W1mWm.T])
        .astype(np.float32)
        .astype(bf)
        .reshape(4, 128, 2 * D)
    )
    w2T = np.ascontiguousarray(W2.T).astype(bf).reshape(4, 128, D)
    bias = np.stack(
        [bq.reshape(D).astype(np.float32), bk.reshape(D).astype(np.float32),
         b2.reshape(D)], axis=1
    ).reshape(2, 128, 3)
    shared = {
        "wqT": wqT,
        "wkT": wkT,
        "wvT": wvT,
        "w1T": np.ascontiguousarray(w1T),
        "w2T": w2T,
        "bias": np.ascontiguousarray(bias),
        "bv": np.ascontiguousarray(bv.reshape(1, D)).astype(bf),
    }
    in_maps = []
    for b in range(B):
        m = dict(shared)
        m["x"] = np.ascontiguousarray(x[b]).astype(bf).reshape(2, 128, N)
        m["src"] = np.ascontiguousarray(source[b]).astype(bf).reshape(2, 128, M)
        in_maps.append(m)

    nc = _get_nc()
    res = run_bass_kernel_spmd(nc, in_maps, core_ids=list(range(B)))
    return np.stack([res.results[b]["out"] for b in range(B)], axis=0)


# revision 25
# speedup vs baseline: 1.1858x; 1.0192x over previous
"""AttentionalPropagation (SuperGlue-style GNN message passing) on 8 TRN2 NeuronCores.

Sharding: pure data parallel over the batch dim (B=8 -> one batch element per core).
Per-core computation (x, src are (256, 2048) slices; all matmuls in bf16, f32 accum):

  Q = WqS @ x + bq          (256, 2048)   stacked-head layout, c = h*64+dh
  K = WkS @ s + bk          (256, 2048)
  VT = s^T @ WvS^T + bv     (2048, 256)   keys on partitions (transposed layout)
  per head h: S^T[m,n] = K_h[:,m] . Q_h[:,n]  -> exp(S^T/8)  (no max-subtraction;
      scores are O(1) so exp is safe)
  msg_u[dh,n] = sum_m exp . VT[m, h*64+dh]  (col-packed head pairs)
  den[n] = sum_m exp                        (4-way col-packed ones-matmuls)
  msg = msg_u / den
  h1 = W1x @ x + (W1m@WmP) @ msg   (Wm folded into W1 on host; b1/bm-terms cancel
                                    in InstanceNorm)
  hn = relu(h1 - mean);  out = (W2 * rstd) @ hn + b2   (rstd>0 commutes with relu)

Scheduling: software-pipelined one n-chunk back AND interleaved at super-tile
granularity (scores for chunk j alternate with msg/den for chunk j-1 in the PE
stream), m-accumulation chains run reversed so Tile emits at most one semaphore
wait per chain.
"""

import os
import sys

for _p in ("/opt/trn_rl_repo",):
    if _p not in sys.path:
        sys.path.insert(0, _p)

import numpy as np
import ml_dtypes

import concourse.bass as bass
import concourse.mybir as mybir
from concourse import bacc
from concourse import library_config
from concourse.bass import ts
from concourse.tile import TileContext
from concourse.bass_utils import run_bass_kernel_spmd

F32 = mybir.dt.float32
BF16 = mybir.dt.bfloat16
AF = mybir.ActivationFunctionType
ALU = mybir.AluOpType

B, D, N, M, H, DH = 8, 256, 2048, 2048, 4, 64
EPS = 1e-5
NCH = 4  # n-chunks of 512
CHUNK = 512


def _build():
    nc = bacc.Bacc("TRN2", target_bir_lowering=False, debug=False, num_devices=8)

    x_d = nc.dram_tensor("x", [2, 128, N], BF16, kind="ExternalInput").ap()
    s_d = nc.dram_tensor("src", [2, 128, M], BF16, kind="ExternalInput").ap()
    wq_d = nc.dram_tensor("wqT", [2, 128, D], BF16, kind="ExternalInput").ap()
    wk_d = nc.dram_tensor("wkT", [2, 128, D], BF16, kind="ExternalInput").ap()
    wv_d = nc.dram_tensor("wvT", [2, 128, D], BF16, kind="ExternalInput").ap()
    w1_d = nc.dram_tensor("w1T", [4, 128, 2 * D], BF16, kind="ExternalInput").ap()
    w2_d = nc.dram_tensor("w2T", [4, 128, D], BF16, kind="ExternalInput").ap()
    # biases packed as columns: [bq, bk, b2]
    bias_d = nc.dram_tensor("bias", [2, 128, 3], F32, kind="ExternalInput").ap()
    bv_d = nc.dram_tensor("bv", [1, D], BF16, kind="ExternalInput").ap()
    out_d = nc.dram_tensor("out", [D, N], F32, kind="ExternalOutput").ap()

    with TileContext(nc) as tc:
        nc.gpsimd.load_library(library_config.attn)
        with (
            tc.tile_pool(name="const", bufs=1) as const,
            tc.tile_pool(name="data", bufs=1) as data,
            tc.tile_pool(name="reuse", bufs=2) as reuse,
            tc.tile_pool(name="exps", bufs=6) as exps,
            tc.tile_pool(name="small", bufs=2) as small,
            tc.tile_pool(name="msgn", bufs=4) as msgn,
            tc.tile_pool(name="ps_sc", bufs=2, space="PSUM") as ps_sc,
            tc.tile_pool(name="ps_shared", bufs=4, space="PSUM") as ps_shared,
        ):
            # ---- inputs + weights (few large DMAs; x/wq first for fast start) ----
            x_sb = data.tile([128, 2, N], BF16, name="x")
            wq_sb = const.tile([128, 2, D], BF16, name="wq")
            nc.sync.dma_start(out=x_sb[:], in_=x_d.rearrange("k p n -> p k n"))
            nc.sync.dma_start(out=wq_sb[:], in_=wq_d.rearrange("k p n -> p k n"))
            s_sb = reuse.tile([128, 2, M], BF16, name="s", tag="big")
            wk_sb = const.tile([128, 2, D], BF16, name="wk")
            wv_sb = const.tile([128, 2, D], BF16, name="wv")
            nc.sync.dma_start(out=s_sb[:], in_=s_d.rearrange("k p n -> p k n"))
            nc.sync.dma_start(out=wk_sb[:], in_=wk_d.rearrange("k p n -> p k n"))
            nc.sync.dma_start(out=wv_sb[:], in_=wv_d.rearrange("k p n -> p k n"))
            bias_sb = const.tile([128, 2, 3], F32, name="bias")
            nc.sync.dma_start(out=bias_sb[:], in_=bias_d.rearrange("k p n -> p k n"))
            bv_bc = const.tile([128, D], BF16, name="bvbc")
            bv_src = bass.AP(
                tensor=bv_d.tensor, offset=bv_d.offset, ap=[[0, 128]] + bv_d.ap[1:]
            )
            nc.sync.dma_start(out=bv_bc[:], in_=bv_src)
            w1_sb = const.tile([128, 4, 2 * D], BF16, name="w1")
            nc.sync.dma_start(out=w1_sb[:], in_=w1_d.rearrange("k p n -> p k n"))
            w2_sb = const.tile([128, 4, D], BF16, name="w2")
            nc.sync.dma_start(out=w2_sb[:], in_=w2_d.rearrange("k p n -> p k n"))
            eps_sb = const.tile([128, 1], F32, name="eps")
            nc.vector.memset(eps_sb[:], EPS)
            ones_sb = const.tile([128, 1], BF16, name="ones")
            nc.vector.memset(ones_sb[:], 1.0)

            # ---- QKV projections (weight-stationary: 1 LDW per 4 MMs) ----
            q_sb = data.tile([128, 2, N], BF16, name="q")
            k_sb = data.tile([128, 2, M], BF16, name="k")

            def emit_qk(c):
                for dst, w_sb, src_t, b_col in (
                    (q_sb, wq_sb, x_sb, 0),
                    (k_sb, wk_sb, s_sb, 1),
                ):
                    ps = [
                        ps_sc.tile([128, 2, CHUNK], F32, name="qk", tag="scps")
                        for _ in range(2)
                    ]
                    for k in range(2):
                        for j in range(NCH):
                            nc.tensor.matmul(
                                ps[j // 2][:, j % 2, :],
                                w_sb[:, k, ts(c, 128)],
                                src_t[:, k, ts(j, CHUNK)],
                                start=(k == 0),
                                stop=(k == 1),
                            )
                    for half in range(2):
                        nc.vector.tensor_scalar_add(
                            dst[:, c, ts(half, 2 * CHUNK)],
                            ps[half][:],
                            bias_sb[:, c, b_col : b_col + 1],
                        )

            # V^T: (m, c) layout, 65-wide per-head blocks with a ones column
            vT_sb = [data.tile([128, H, DH + 1], BF16, name=f"vT{t}")
                     for t in range(16)]

            def emit_vT(trange):
                for t in trange:
                    vp = ps_shared.tile([128, D], F32, name="vps", tag="sps")
                    for k in range(2):
                        nc.tensor.matmul(
                            vp[:],
                            s_sb[:, k, ts(t, 128)],
                            wv_sb[:, k, :],
                            start=(k == 0),
                            stop=(k == 1),
                        )
                    nc.vector.tensor_add(
                        vT_sb[t][:, :, 0:DH],
                        vp[:].rearrange("p (h d) -> p h d", h=H),
                        bv_bc[:].rearrange("p (h d) -> p h d", h=H),
                    )
                    nc.vector.memset(vT_sb[t][:, :, DH : DH + 1], 1.0)

            # ---- attention ----
            h1_sb = data.tile([128, 4, N], BF16, name="h1")
            stats_sb = data.tile([128, 4, NCH, 6], F32, name="stats")
            eS = {}  # (j, h, half) -> expS tile (128, 8, CHUNK)
            mn = {}  # (j, p) -> normalized msg pair tile (128, CHUNK)
            mps = {}  # (j, p) -> msg psum ; (j, 'd') -> den psum

            def emit_scores_super(j, p, s):
                # scores + exp for super-tile s (m-tiles 2s, 2s+1), head pair p
                if s == 0:
                    for h2 in range(2):
                        eS[(j, 2 * p + h2)] = exps.tile(
                            [128, 16, CHUNK], BF16, name="expS", tag="expS"
                        )
                scp = [
                    ps_sc.tile([128, 2, CHUNK], F32, name="sc", tag="scps")
                    for _ in range(2)
                ]
                for jj in range(2):
                    mt = 2 * s + jj
                    for h2 in range(2):
                        nc.tensor.matmul(
                            scp[h2][:, jj, :],
                            k_sb[ts(h2, DH), p, ts(mt, 128)],
                            q_sb[ts(h2, DH), p, ts(j, CHUNK)],
                            start=True,
                            stop=True,
                        )
                for h2 in range(2):
                    nc.scalar.activation(
                        eS[(j, 2 * p + h2)][:, 2 * s : 2 * s + 2, :],
                        scp[h2][:],
                        AF.Exp,
                        scale=1.0 / 8.0,
                    )

            def emit_msg_head(j, h):
                # augmented-V msg chain (psum row 64 = denominator)
                p, h2 = h // 2, h % 2
                if h2 == 0:
                    mn[(j, p)] = msgn.tile([128, CHUNK], BF16, name="mn", tag="mn")
                mp = ps_shared.tile([DH + 1, CHUNK], F32, name="msgps", tag="sps")
                for mt in range(16):
                    nc.tensor.matmul(
                        mp[:],
                        vT_sb[mt][:, h, :],
                        eS[(j, h)][:, mt, :],
                        start=(mt == 0),
                        stop=(mt == 15),
                    )
                del eS[(j, h)]
                den = small.tile([1, CHUNK], F32, name="den", tag="den")
                nc.vector.tensor_copy(den[:], mp[DH : DH + 1, :])
                rden = small.tile([1, CHUNK], F32, name="rden", tag="rden")
                nc.vector.reciprocal_approx_fast(rden[:], den[:])
                rbc = small.tile([DH, CHUNK], F32, name="rbc", tag="rbc")
                nc.gpsimd.partition_broadcast(rbc[:], rden[:])
                nc.vector.tensor_mul(mn[(j, p)][ts(h2, DH), :], mp[0:DH, :], rbc[:])

            def emit_norm_h1(j):
                # h1 = W1x @ x + W1mWm @ msg
                for o in range(4):
                    hp = ps_shared.tile([128, CHUNK], F32, name="h1ps", tag="sps")
                    for k in range(4):
                        rhs = (
                            x_sb[:, k, ts(j, CHUNK)] if k < 2 else mn[(j, k - 2)][:]
                        )
                        nc.tensor.matmul(
                            hp[:],
                            w1_sb[:, k, ts(o, 128)],
                            rhs,
                            start=(k == 0),
                            stop=(k == 3),
                        )
                    nc.vector.tensor_copy(h1_sb[:, o, ts(j, CHUNK)], hp[:])
                    nc.vector.bn_stats(
                        stats_sb[:, o, j, :], h1_sb[:, o, ts(j, CHUNK)]
                    )

            # ---- schedule ----
            emit_qk(0)
            for s in range(8):
                emit_scores_super(0, 0, s)
            emit_qk(1)
            for s in range(8):
                emit_scores_super(0, 1, s)
                emit_vT(range(2 * s, 2 * s + 2))
            for j in range(1, NCH - 1):
                for s in range(8):
                    emit_scores_super(j, 0, s)
                    emit_scores_super(j, 1, s)
                for h in range(4):
                    emit_msg_head(j - 1, h)
                emit_norm_h1(j - 1)
            jL = NCH - 1
            for s in range(8):
                emit_scores_super(jL, 0, s)
            for s in range(8):
                emit_scores_super(jL, 1, s)
            for h in range(4):
                emit_msg_head(jL - 1, h)
            emit_norm_h1(jL - 1)
            for h in range(4):
                emit_msg_head(jL, h)
            emit_norm_h1(jL)

            # ---- InstanceNorm (relu on DVE, rstd folded into W2) + W2 ----
            hn_sb = reuse.tile([128, 4, N], BF16, name="hn", tag="big")
            mean = small.tile([128, 4], F32, name="mean", tag="mean")
            for o in range(4):
                mv = small.tile([128, 2], F32, name="mv", tag="mv")
                nc.vector.bn_aggr(mv[:], stats_sb[:, o, :, :])
                nc.vector.tensor_copy(mean[:, o : o + 1], mv[:, 0:1])
                std = small.tile([128, 1], F32, name="std", tag="std")
                nc.scalar.activation(std[:], mv[:, 1:2], AF.Sqrt, bias=eps_sb[:])
                rstd = small.tile([128, 1], F32, name="rstd", tag="rstd")
                nc.vector.reciprocal(rstd[:], std[:])
                nc.vector.tensor_scalar_mul(w2_sb[:, o, :], w2_sb[:, o, :], rstd[:])
            for j in range(NCH):
                for o in range(4):
                    nc.vector.tensor_scalar(
                        hn_sb[:, o, ts(j, CHUNK)],
                        h1_sb[:, o, ts(j, CHUNK)],
                        mean[:, o : o + 1],
                        0.0,
                        op0=ALU.subtract,
                        op1=ALU.max,
                    )
                for c in range(2):
                    op = ps_shared.tile([128, CHUNK], F32, name="ops", tag="sps")
                    for ki, k in enumerate((3, 2, 1, 0)):
                        nc.tensor.matmul(
                            op[:],
                            w2_sb[:, k, ts(c, 128)],
                            hn_sb[:, k, ts(j, CHUNK)],
                            start=(ki == 0),
                            stop=(ki == 3),
                        )
                    ot = small.tile([128, CHUNK], F32, name="outt", tag="outt")
                    nc.vector.tensor_scalar_add(ot[:], op[:], bias_sb[:, c, 2:3])
                    nc.sync.dma_start(out=out_d[ts(c, 128), ts(j, CHUNK)], in_=ot[:])

    nc.compile()
    return nc


_NC = None


def _get_nc():
    global _NC
    if _NC is None:
        _NC = _build()
    return _NC


def kernel(**inputs):
    x = np.asarray(inputs["x"], np.float32)
    source = np.asarray(inputs["source"], np.float32)
    Wq = np.asarray(inputs["Wq"], np.float32)
    bq = np.asarray(inputs["bq"], np.float32)
    Wk = np.asarray(inputs["Wk"], np.float32)
    bk = np.asarray(inputs["bk"], np.float32)
    Wv = np.asarray(inputs["Wv"], np.float32)
    bv = np.asarray(inputs["bv"], np.float32)
    Wm = np.asarray(inputs["Wm"], np.float64)
    W1 = np.asarray(inputs["W1"], np.float64)
    W2 = np.asarray(inputs["W2"], np.float32)
    b2 = np.asarray(inputs["b2"], np.float32)

    bf = ml_dtypes.bfloat16
    wqT = np.ascontiguousarray(Wq.reshape(H * DH, D).T).astype(bf).reshape(2, 128, D)
    wkT = np.ascontiguousarray(Wk.reshape(H * DH, D).T).astype(bf).reshape(2, 128, D)
    wvT = np.ascontiguousarray(Wv.reshape(H * DH, D).T).astype(bf).reshape(2, 128, D)
    # message-channel permutation (dh-major -> head-major) folded into Wm
    WmP = Wm.reshape(D, DH, H).transpose(0, 2, 1).reshape(D, D)
    # fold Wm into W1's message half; b1 and W1m@bm cancel in InstanceNorm
    W1mWm = W1[:, D:] @ WmP
    w1T = (
        np.vstack([W1[:, :D].T, W1mWm.T])
        .astype(np.float32)
        .astype(bf)
        .reshape(4, 128, 2 * D)
    )
    w2T = np.ascontiguousarray(W2.T).astype(bf).reshape(4, 128, D)
    bias = np.stack(
        [bq.reshape(D).astype(np.float32), bk.reshape(D).astype(np.float32),
         b2.reshape(D)], axis=1
    ).reshape(2, 128, 3)
    shared = {
        "wqT": wqT,
        "wkT": wkT,
        "wvT": wvT,
        "w1T": np.ascontiguousarray(w1T),
        "w2T": w2T,
        "bias": np.ascontiguousarray(bias),
        "bv": np.ascontiguousarray(bv.reshape(1, D)).astype(bf),
    }
    in_maps = []
    for b in range(B):
        m = dict(shared)
        m["x"] = np.ascontiguousarray(x[b]).astype(bf).reshape(2, 128, N)
        m["src"] = np.ascontiguousarray(source[b]).astype(bf).reshape(2, 128, M)
        in_maps.append(m)

    nc = _get_nc()
    res = run_bass_kernel_spmd(nc, in_maps, core_ids=list(range(B)))
    return np.stack([res.results[b]["out"] for b in range(B)], axis=0)
